# revision 13
# baseline (speedup 1.0000x reference)
"""CenterNet loss on 8 Trainium2 NeuronCores.

Strategy (pure data parallel, hint-aligned): batch dim B=16 is sharded
2-per-core across 8 cores. The dense, memory-bound part of the loss --
sum over all B*C*H*W cls_pred elements of p^2 * ln(1 - p) -- streams
through each core as a raw-bass 5-engine pipeline, fed with *bfloat16*
inputs (host-side cast) so HBM traffic halves (5.24 MB/core) and the DVE
runs in its 2x 16-bit mode:

    sync:   all input DMAs queued up front (whole shard fits in SBUF:
            40 KB/partition per buffer, 4 buffers = 160 KB of ~208 KB)
    scalar: L = Ln(1 - q)          bf16 -> bf16   (the only ACT-capable op;
            ~17.1 us at 1 elem/cycle/lane -- the pipeline's critical engine)
    vector: s = q*q (2x bf16), prod = s*L (2x bf16); tail tiles fuse
            prod+reduce via scalar_tensor_tensor accum_out
    gpsimd: a share of the squares (engine balancing)
    tensor: psum[1,512] += ones.T @ prod  (reduction; dummy matmuls at
            start warm the PE HAM clock gate from 1.2 to 2.4 GHz)

Host-side (exact, touches only gt_* plus a few thousand gathered values):
  * bf16 cast: values rounding to 1.0 (p >= 0.998046875) are sent as 0.0
    (device term is exactly 0) and their reference term added on host
  * focal-loss corrections at the <=450 gaussian-heatmap pixels per batch
  * the top-CAND-smallest window mask per batch and its offset/size L1 sums
Device approximations (bf16 rounding of p and intermediates; ACT spline
Ln) contribute < ~5e-4 relative on the loss; tolerance is 2e-2."""

import os

import numpy as np

B, C, H, W = 16, 80, 128, 128
N, CAND = 50, 100
N_CORES = 8
BATCH_PER_CORE = B // N_CORES
ONE_V = float(np.exp(-0.5))
TWO_V = float(np.exp(-1.0))
F32 = np.float32

P = 128
COLS = (BATCH_PER_CORE * C * H * W) // P  # 20480 bf16 columns per core

# ---- pipeline chunking (columns; all cumulative boundaries line up) ----
# DMA chunks: small first (fast compute start), small last (short tail).
# Each chunk gets a dedicated semaphore: "chunk b's sem == 16" plus the
# SDMA engines' per-engine FIFO order guarantees all chunks <= b landed
# (a single shared counter is racy across queued DMAs).
DMA_CHUNKS = [2048, 4096, 4096, 4096, 4096, 2048]
# ACT Ln chunks: few ops (per-op overhead ~190 ns); first two share one
# DMA receipt, later ones lag one fat chunk behind the stream.
LN_CHUNKS = [1024, 1024, 4096, 4096, 4096, 4096, 1024, 1024]
# compute units: product + reduction granularity
UNITS = [2048] * 9 + [1024, 512, 512]
# tail units computed wholly on DVE (sq + fused prod/reduce via STT
# accum_out) so the PE/diag chain stays off the kernel exit path
DVE_TAIL = ()
MMW = 128           # PE stationary width (diag-trick chunk)
N_WARM_MM = 8       # dummy matmuls to warm the PE HAM clock gate

_BASS_CACHE = {}


def _cum(chunks):
    out = []
    t = 0
    for c in chunks:
        t += c
        out.append(t)
    return out


def _build_raw():
    """Raw-bass (no TileContext) pipeline over bf16 input x[128, COLS].

    Dense sum via the matmul-diagonal trick: DVE computes m = q * Ln(1-q)
    (one 2x bf16 pass); PE accumulates acc128 += q_chunk.T @ m_chunk over
    all 128-col chunks, whose diagonal is diag[p] = sum q^2 Ln(1-q) over
    that partition's columns; one STT against the identity extracts it."""
    from contextlib import ExitStack

    import concourse.bass as bass
    from concourse import mybir

    f32 = mybir.dt.float32
    b16 = mybir.dt.bfloat16
    AF = mybir.ActivationFunctionType
    OP = mybir.AluOpType

    dma_cum = _cum(DMA_CHUNKS)
    ln_cum = _cum(LN_CHUNKS)
    unit_cum = _cum(UNITS)
    assert dma_cum[-1] == ln_cum[-1] == unit_cum[-1] == COLS

    def dma_chunk_of(col):
        # index of the last DMA chunk needed for cols [0, col) to be valid
        return next(i for i, c in enumerate(dma_cum) if c >= col)

    def ln_target(col):
        return next(i + 1 for i, c in enumerate(ln_cum) if c >= col)

    nu = len(UNITS)
    pe_units = [u for u in range(nu) if u not in DVE_TAIL]
    n_out = 1 + len(DVE_TAIL)  # diag column + one per tail unit

    nc = bass.Bass("TRN2", target_bir_lowering=False, debug=False)
    x = nc.dram_tensor("x", [P, COLS], b16, kind="ExternalInput")
    ident = nc.dram_tensor("ident", [P, MMW], f32, kind="ExternalInput")
    out2 = nc.dram_tensor("out2", [P, n_out], f32, kind="ExternalOutput")

    with ExitStack() as ctx:
        ent = ctx.enter_context
        xt = ent(nc.sbuf_tensor("xt", [P, COLS], b16))
        lt = ent(nc.sbuf_tensor("lt", [P, COLS], b16))
        mt = ent(nc.sbuf_tensor("mt", [P, COLS], b16))
        st = ent(nc.sbuf_tensor("st", [P, 2048], b16))   # tail squares
        idn = ent(nc.sbuf_tensor("idn", [P, MMW], f32))
        dscr = ent(nc.sbuf_tensor("dscr", [P, MMW], f32))  # diag STT out
        ones = ent(nc.sbuf_tensor("ones", [P, 512], b16))
        warm = ent(nc.sbuf_tensor("warm", [P, 1], b16))
        acc2 = ent(nc.sbuf_tensor("acc2", [P, n_out], f32))
        acc128 = ent(nc.psum_tensor("acc128", [P, MMW], f32))
        accd = ent(nc.psum_tensor("accd", [1, 512], f32))

        csem = [ent(nc.semaphore(name=f"c{i}")) for i in range(len(DMA_CHUNKS))]
        ident_sem = ent(nc.semaphore(name="ident_sem"))
        ones_sem = ent(nc.semaphore(name="ones_sem"))
        ln_sem = ent(nc.semaphore(name="ln_sem"))     # +1 per Ln chunk
        dve_sem = ent(nc.semaphore(name="dve_sem"))   # +1 per DVE prod/STT
        pe_sem = ent(nc.semaphore(name="pe_sem"))     # +1 after last matmul
        odma_sem = ent(nc.semaphore(name="odma_sem"))

        with nc.Block() as block:

            @block.sync
            def _(sync):
                off = 0
                for i, c in enumerate(DMA_CHUNKS):
                    sync.dma_start(
                        xt[:, off : off + c], x[:, off : off + c]
                    ).then_inc(csem[i], 16)
                    off += c
                sync.dma_start(idn[:], ident[:]).then_inc(ident_sem, 16)
                # all DVE incs done (prods + tail STTs + diag STT)
                sync.wait_ge(dve_sem, nu + 1)
                sync.dma_start(out2[:], acc2[:]).then_inc(odma_sem, 16)
                # no wait on odma_sem: the 1.5 KB write's ~2.5 us HBM
                # receipt drains under the NEFF postamble; NRT completes
                # the execution only after DMA rings are idle.

            @block.scalar
            def _(scalar):
                # dummy Ln fires the ACT table load during the DMA ramp
                scalar.wait_ge(ones_sem, 1)
                scalar.activation(warm[:], ones[:, :1], AF.Ln)
                off = 0
                for c in LN_CHUNKS:
                    scalar.wait_ge(csem[dma_chunk_of(off + c)], 16)
                    scalar.activation(
                        lt[:, off : off + c],
                        xt[:, off : off + c],
                        AF.Ln,
                        bias=1.0,
                        scale=-1.0,
                    ).then_inc(ln_sem, 1)
                    off += c

            @block.vector
            def _(vector):
                vector.memset(ones[:], 1.0).then_inc(ones_sem, 1)
                tail_base = unit_cum[DVE_TAIL[0] - 1] if DVE_TAIL else 0
                tails = []
                for k, u in enumerate(DVE_TAIL):
                    lo = unit_cum[u] - UNITS[u]
                    hi = unit_cum[u]
                    tails.append((k, u, lo, hi, lo - tail_base))
                # m = q * Ln(1-q) per PE-path unit (2x bf16); tail squares
                # hoisted before the last prod (they only need their DMA)
                for i, u in enumerate(pe_units):
                    if i == len(pe_units) - 1:
                        for k, tu, lo, hi, so in tails:
                            vector.wait_ge(csem[dma_chunk_of(hi)], 16)
                            vector.tensor_mul(
                                st[:, so : so + UNITS[tu]],
                                xt[:, lo:hi],
                                xt[:, lo:hi],
                            )
                    lo = unit_cum[u] - UNITS[u]
                    hi = unit_cum[u]
                    vector.wait_ge(ln_sem, ln_target(hi))
                    vector.tensor_mul(
                        mt[:, lo:hi], xt[:, lo:hi], lt[:, lo:hi]
                    ).then_inc(dve_sem, 1)
                # tail fused prod+reduce (accum_out)
                for k, u, lo, hi, so in tails:
                    vector.wait_ge(ln_sem, ln_target(hi))
                    vector.scalar_tensor_tensor(
                        out=mt[:, lo:hi],
                        in0=st[:, so : so + UNITS[u]],
                        scalar=1.0,
                        in1=lt[:, lo:hi],
                        op0=OP.mult,
                        op1=OP.mult,
                        accum_out=acc2[:, 1 + k : 2 + k],
                    ).then_inc(dve_sem, 1)
                # diagonal of the PE accumulator -> acc2[:, 0]
                vector.wait_ge(pe_sem, 1)
                vector.wait_ge(ident_sem, 16)
                vector.scalar_tensor_tensor(
                    out=dscr[:],
                    in0=acc128[:],
                    scalar=1.0,
                    in1=idn[:],
                    op0=OP.mult,
                    op1=OP.mult,
                    accum_out=acc2[:, 0:1],
                ).then_inc(dve_sem, 1)

            @block.tensor
            def _(tensor):
                tensor.wait_ge(ones_sem, 1)
                # dummy matmuls warm the HAM clock gate (1.2 -> 2.4 GHz)
                for _ in range(N_WARM_MM):
                    tensor.matmul(
                        accd[:], ones[:, :1], ones[:], start=True, stop=True
                    )
                first = True
                for i, u in enumerate(pe_units):
                    lo = unit_cum[u] - UNITS[u]
                    tensor.wait_ge(dve_sem, i + 1)
                    for j in range(UNITS[u] // MMW):
                        last = u == pe_units[-1] and j == UNITS[u] // MMW - 1
                        a = lo + j * MMW
                        mm = tensor.matmul(
                            acc128[:],
                            xt[:, a : a + MMW],
                            mt[:, a : a + MMW],
                            start=first,
                            stop=last,
                        )
                        first = False
                        if last:
                            mm.then_inc(pe_sem, 1)

    return nc


def _get_bass():
    if "nc" not in _BASS_CACHE:
        _BASS_CACHE["nc"] = _build_raw()
    return _BASS_CACHE["nc"]


# ----------------------------------------------------------------------------
# Host-side bf16 preparation
# ----------------------------------------------------------------------------

def _bf16_prep(cls_pred_f32):
    """Round f32 -> bf16 (RTNE). Values that round to 1.0 are replaced with
    0.0 (device contributes exactly 0 for them) and returned as a host-side
    correction sum of their reference negative-term. Returns (bits_u16,
    tail_correction)."""
    flat = np.ascontiguousarray(cls_pred_f32, dtype=np.float32).reshape(-1)
    u = flat.view(np.uint32)
    bits = ((u + 0x7FFF + ((u >> 16) & 1)) >> 16).astype(np.uint16)
    tail = bits == 0x3F80  # rounded to 1.0  <=>  p >= 0.998046875
    if tail.any():
        p = np.minimum(flat[tail].astype(np.float64), 0.9999)
        corr = float(np.sum(p * p * np.log1p(-p)))
        bits[tail] = 0
    else:
        corr = 0.0
    return bits, corr


def _run_device(cls_pred, trace=False):
    """Returns (dense_neg_sum, BassKernelResults)."""
    import ml_dtypes
    from concourse.bass_utils import run_bass_kernel_spmd

    nc = _get_bass()
    bits, tail_corr = _bf16_prep(cls_pred)
    bits = bits.reshape(B, -1)
    ident = np.eye(P, MMW, dtype=np.float32)
    in_maps = []
    for i in range(N_CORES):
        shard = bits[i * BATCH_PER_CORE : (i + 1) * BATCH_PER_CORE]
        shard = np.ascontiguousarray(shard).reshape(P, COLS)
        in_maps.append({"x": shard.view(ml_dtypes.bfloat16), "ident": ident})
    res = run_bass_kernel_spmd(
        nc, in_maps, core_ids=list(range(N_CORES)), trace=trace
    )
    dense = tail_corr
    for r in res.results:
        dense += np.asarray(r["out2"], dtype=np.float64).sum()
    return dense, res


# ----------------------------------------------------------------------------
# Host-side sparse parts (depend only on gt_box/gt_class + a few thousand
# gathered prediction values).
# ----------------------------------------------------------------------------

def _heatmap_points(gt_box, gt_class):
    """Per-batch {(c, x, y): g} replicating _cls_gt's scatter-max heatmap."""
    gt_box = gt_box.astype(F32)
    gt_class_i = gt_class.astype(np.int64)
    out = []
    for b in range(B):
        pts = {}
        w = gt_box[b, :, 2] - gt_box[b, :, 0]
        h = gt_box[b, :, 3] - gt_box[b, :, 1]
        cx = np.floor_divide(np.floor_divide(w, F32(2.0)), F32(4.0)).astype(np.int32)
        cy = np.floor_divide(np.floor_divide(h, F32(2.0)), F32(4.0)).astype(np.int32)
        ch = np.maximum(gt_class_i[b], 0).astype(np.int32)
        valid = gt_class_i[b] != -1
        interior = valid & (cx >= 1) & (cy >= 1) & (cx + 1 < H) & (cy + 1 < W)
        for n in range(N):
            if valid[n]:
                k = (int(ch[n]), int(cx[n]), int(cy[n]))
                # XLA scatter drops out-of-bounds updates (center is unclipped)
                if 0 <= k[1] < H and 0 <= k[2] < W:
                    pts[k] = max(pts.get(k, 0.0), 1.0)
            if interior[n]:
                for dx, dy, v in (
                    (-1, -1, TWO_V), (-1, 0, ONE_V), (-1, 1, TWO_V),
                    (0, -1, ONE_V), (0, 1, ONE_V),
                    (1, -1, TWO_V), (1, 0, ONE_V), (1, 1, TWO_V),
                ):
                    x = int(np.clip(cx[n] + dx, 0, H - 1))
                    y = int(np.clip(cy[n] + dy, 0, W - 1))
                    k2 = (int(ch[n]), x, y)
                    cur = pts.get(k2, 0.0)
                    if v > cur:
                        pts[k2] = v
        out.append(pts)
    return out


def _focal_correction(cls_pred, gt_box, gt_class):
    """Sum over heatmap pixels of (reference term - plain negative term).

    The device sums p^2*log(1-p) over every pixel; at a pixel whose heatmap
    value is g the reference instead uses (1-p)^4*log(p) when g == 1, or
    (1-g)^4 * p^2 * log(1-p) otherwise."""
    delta = 0.0
    for b, pts in enumerate(_heatmap_points(gt_box, gt_class)):
        for (c, x, y), g in pts.items():
            p = float(np.clip(cls_pred[b, c, x, y], 1e-4, 0.9999))
            neg = p * p * np.log1p(-p)
            if g == 1.0:
                delta += (1.0 - p) ** 4 * np.log(p) - neg
            else:
                delta += ((1.0 - g) ** 4 - 1.0) * neg
    return delta


def _mask_losses(cls_pred, offset_pred, size_pred, gt_box, gt_class):
    """Replicates _target_one (top-CAND smallest in the last box's window)
    and the masked offset/size L1 sums. Returns (off_sum, size_sum, num_pos).
    """
    gt_box = gt_box.astype(F32)
    gt_class_i = gt_class.astype(np.int64)
    off_sum = 0.0
    size_sum = 0.0
    num_pos = 0
    for b in range(B):
        valid = gt_class_i[b] != -1
        last = max(int(np.where(valid, np.arange(N), -1).max()), 0)
        if not bool(valid.any()):
            continue
        box = gt_box[b, last]
        ch = int(max(int(gt_class_i[b, last]), 0))
        wv = F32(box[2]) - F32(box[0])
        hv = F32(box[3]) - F32(box[1])
        cx = int(np.floor_divide(np.floor_divide(wv, F32(2.0)), F32(4.0)))
        cy = int(np.floor_divide(np.floor_divide(hv, F32(2.0)), F32(4.0)))
        w4 = int(np.floor_divide(wv, F32(4.0)))
        h4 = int(np.floor_divide(hv, F32(4.0)))
        left = max((cx - w4 // 2) // 2, 0)
        right = min((cx + w4 // 2) // 2, H // 2)
        top = max((cy - h4 // 2) // 2, 0)
        bottom = min((cy + h4 // 2) // 2, W // 2)
        if right <= left or bottom <= top:
            continue
        flat = cls_pred[b, ch, left:right, top:bottom].reshape(-1)
        k = min(CAND, flat.size)
        # jax.lax.top_k(-vals, CAND) is stable (ties -> lower index first);
        # window row-major order matches global row-major order, so a stable
        # ascending argsort over the window selects the identical pixel set.
        order = np.argsort(flat, kind="stable")[:k]
        wi = order // (bottom - top) + left
        wj = order % (bottom - top) + top
        num_pos += k
        cxf = wv / F32(2.0) / F32(4.0)
        cyf = hv / F32(2.0) / F32(4.0)
        off0 = float(cxf - np.floor(cxf))
        off1 = float(cyf - np.floor(cyf))
        po = offset_pred[b]
        ps = size_pred[b]
        off_sum += np.abs(po[0, wi, wj].astype(np.float64) - off0).sum()
        off_sum += np.abs(po[1, wi, wj].astype(np.float64) - off1).sum()
        size_sum += np.abs(ps[0, wi, wj].astype(np.float64) - float(wv)).sum()
        size_sum += np.abs(ps[1, wi, wj].astype(np.float64) - float(hv)).sum()
    return off_sum, size_sum, max(num_pos, 1)


def _combine(dense, cls_pred, offset_pred, size_pred, gt_box, gt_class):
    delta = _focal_correction(cls_pred, gt_box, gt_class)
    off_sum, size_sum, num_pos = _mask_losses(
        cls_pred, offset_pred, size_pred, gt_box, gt_class
    )
    cls_loss = -(dense + delta) / (B * H * W)
    offset_loss = off_sum / num_pos
    size_loss = size_sum / num_pos
    return cls_loss + 0.1 * size_loss + 1.0 * offset_loss


def kernel_with_results(
    cls_pred, offset_pred, size_pred, gt_box, gt_class, trace=False
):
    cls_pred = np.asarray(cls_pred)
    dense, res = _run_device(cls_pred, trace=trace)
    loss = _combine(
        dense,
        cls_pred,
        np.asarray(offset_pred),
        np.asarray(size_pred),
        np.asarray(gt_box),
        np.asarray(gt_class),
    )
    return np.asarray(loss, dtype=np.float32), res


def kernel(cls_pred, offset_pred, size_pred, gt_box, gt_class):
    loss, _ = kernel_with_results(cls_pred, offset_pred, size_pred, gt_box, gt_class)
    return loss
